# revision 11
# baseline (speedup 1.0000x reference)
"""Trainium2 Bass kernel for nn_DifferentiableParallelBeamRadon.

Reference op: parallel-beam Radon transform of image [4,1,256,256] over 180
angles -> sinogram [4,1,180,256] (torch-style affine_grid/grid_sample bilinear
sampling with zeros padding, summed over rotated rows, scaled by 2/255).

D-form strategy
---------------
Geometry is input-independent. For each angle we enumerate the bilinear taps
(row p, detector j, image column x, weight C) of the row-binned operator, fit
the affine band  x ~ al*j + sl*p + c,  and shear each row by t[p] =
floor(sl*p+c) so the band becomes row-independent: sheared column
xh = x - t[p].  Rows are φ-grouped (8 bins of frac(sl*p+c)) so that within a
group every sheared column xh is used by a run of at most S=3 consecutive
detectors starting at jwin[g, xh].  Tap weights become 3 column-indexed
coefficient planes D[s, p, xh] (s = j - jwin[group[p], xh]).

Device (per angle): DMA the sheared fp16 image SH[p,(b,h,xh)] (one image
copy - ~3x less traffic than tap-expanded gathers); multiply against the
SBUF-RESIDENT coefficient planes with one scalar_tensor_tensor (4x DVE mode);
reduce rows with 24 small matmuls whose lhsT are per-half φ-group masks
(output rows = (h,s)-block x 8 groups, M=8 costs the same as M=1); DMA the
PSUM T[g,s,b,xh] partial sinograms to DRAM.  Host finishes with per-angle
bincounts:  sino[b, jwin[g,xh]+s] += T[g,s,b,xh].

Coefficients and masks are loaded once in the program preamble (before the
timing loop) and stay resident in SBUF - a weights-stationary serving model.
"""

import os

import numpy as np

IMAGE_SIZE = 256
NUM_ANGLES = 180
NUM_DET = 256
BATCH = 4
N_CORES = 8
G = 8                     # phi groups
N = IMAGE_SIZE

NSLOT = (NUM_ANGLES + N_CORES - 1) // N_CORES  # 23


# ----------------------------------------------------------------------------
# geometry precompute (input independent, cached at import)
# ----------------------------------------------------------------------------

def _angle_tables(a_idx: int):
    """Row-binned tap tables (same derivation as the proven G-form kernel):
    returns (axis, xidx int [256,256] window base, C float64 [4,256,256])."""
    R_MAX = 4
    angles = np.linspace(0.0, 180.0, NUM_ANGLES + 1, dtype=np.float32)[:-1]
    ang = np.deg2rad(angles[a_idx], dtype=np.float32)
    cos = np.cos(ang, dtype=np.float32)
    sin = np.sin(ang, dtype=np.float32)

    j = np.arange(N, dtype=np.float32)
    xs = ((2.0 * j + 1.0) / np.float32(N) - 1.0).astype(np.float32)
    ys = xs.copy()

    gx = (cos * xs[None, :] + sin * ys[:, None]).astype(np.float32)
    gy = (-sin * xs[None, :] + cos * ys[:, None]).astype(np.float32)
    ix = (((gx + 1.0) * np.float32(N) - 1.0) * np.float32(0.5)).astype(np.float32)
    iy = (((gy + 1.0) * np.float32(N) - 1.0) * np.float32(0.5)).astype(np.float32)

    x0 = np.floor(ix)
    y0 = np.floor(iy)
    wx1 = (ix - x0).astype(np.float64)
    wy1 = (iy - y0).astype(np.float64)
    wx0 = 1.0 - wx1
    wy0 = 1.0 - wy1
    x0 = x0.astype(np.int64)
    y0 = y0.astype(np.int64)

    bin_by_row = abs(float(sin)) <= abs(float(cos))

    taps = [
        (y0, x0, wy0 * wx0),
        (y0, x0 + 1, wy0 * wx1),
        (y0 + 1, x0, wy1 * wx0),
        (y0 + 1, x0 + 1, wy1 * wx1),
    ]

    INF = 1 << 20
    qmin = np.full((N, N), INF, dtype=np.int64)
    qmax = np.full((N, N), -INF, dtype=np.int64)
    jj = np.broadcast_to(np.arange(N)[None, :], (N, N))
    binned = []
    for (rr, cc, w) in taps:
        valid = (rr >= 0) & (rr < N) & (cc >= 0) & (cc < N)
        bp, q = (rr, cc) if bin_by_row else (cc, rr)
        m = valid & (w > 0)
        binned.append((bp, q, w, m))
        np.minimum.at(qmin, (bp[m], jj[m]), q[m])
        np.maximum.at(qmax, (bp[m], jj[m]), q[m])

    width = np.where(qmin <= qmax, qmax - qmin + 1, 0)
    assert width.max() <= R_MAX, f"angle {a_idx}: window {width.max()}"
    qbase = np.where(qmin == INF, 0, qmin)

    C = np.zeros((R_MAX, N, N), dtype=np.float64)
    for (bp, q, w, m) in binned:
        r = q[m] - qbase[bp[m], jj[m]]
        np.add.at(C, (r, bp[m], jj[m]), w[m])

    C *= 2.0 / (IMAGE_SIZE - 1)
    return (0 if bin_by_row else 1), qbase.astype(np.int32), C


def _build_angle(a):
    """D-form tables for one angle."""
    axis, xidx, C = _angle_tables(a)
    rr, pp, jj = np.nonzero(np.abs(C) > 0)
    xx = xidx[pp, jj] + rr

    # plane fit x ~ al*j + sl*p + c (robust to edge clipping)
    A = np.stack([jj, pp, np.ones_like(jj)], axis=1).astype(np.float64)
    al, sl, c = np.linalg.lstsq(A, xx.astype(np.float64), rcond=None)[0]
    t = np.floor(sl * np.arange(N) + c).astype(np.int64)
    phi = (sl * np.arange(N) + c) - t
    xh = xx - t[pp]
    xh0 = int(xh.min())
    t += xh0
    xh -= xh0
    EXT = int(xh.max()) + 1

    # phi-group rows (quantiles)
    order = np.argsort(phi, kind="stable")
    group = np.zeros(N, dtype=np.int64)
    for g in range(G):
        group[order[g * N // G : (g + 1) * N // G]] = g

    gg = group[pp]
    jmin = np.full((G, EXT), 1 << 30, dtype=np.int64)
    jmax = np.full((G, EXT), -1, dtype=np.int64)
    np.minimum.at(jmin, (gg, xh), jj)
    np.maximum.at(jmax, (gg, xh), jj)
    used = jmax >= 0
    S = int((jmax - jmin + 1)[used].max())
    jwin = np.where(used, jmin, -1)

    D = np.zeros((S, N, EXT), dtype=np.float64)
    ss = jj - jmin[gg, xh]
    assert ss.min() >= 0 and ss.max() < S
    D[ss, pp, xh] = C[rr, pp, jj]

    return dict(axis=axis, t=t, EXT=EXT, group=group, jwin=jwin, S=S, D=D)


_TABLES = None


def _get_tables():
    """Cached: per-angle D-form tables + slot assignment + combine indices."""
    global _TABLES
    if _TABLES is not None:
        return _TABLES

    tabs = [_build_angle(a) for a in range(NUM_ANGLES)]
    exts = np.array([tb["EXT"] for tb in tabs])
    esses = np.array([tb["S"] for tb in tabs])

    # cost-sorted round-robin: position i -> core i%8, slot i//8
    order = np.argsort(-(esses * exts), kind="stable")
    slot_angle = np.full((NSLOT, N_CORES), -1, dtype=np.int64)
    for i, a in enumerate(order):
        slot_angle[i // N_CORES, i % N_CORES] = a
    ext_slot = np.array(
        [max(tabs[a]["EXT"] for a in row if a >= 0) for row in slot_angle]
    )
    s_slot = np.array(
        [max(tabs[a]["S"] for a in row if a >= 0) for row in slot_angle]
    )

    # combine index per angle: flat bincount index over (g, s, xh)
    SMAX_A = int(esses.max())
    for a, tb in enumerate(tabs):
        E, S_a, jwin = tb["EXT"], tb["S"], tb["jwin"]
        idx = np.full((G, S_a, E), NUM_DET, dtype=np.int64)  # NUM_DET = trash bin
        for g in range(G):
            v = jwin[g] >= 0
            for s in range(S_a):
                idx[g, s, v] = np.minimum(jwin[g, v] + s, NUM_DET)
        tb["cidx"] = idx.ravel()

    _TABLES = (tabs, slot_angle, ext_slot, s_slot)
    return _TABLES


# ----------------------------------------------------------------------------
# bass program (built once, cached)
# ----------------------------------------------------------------------------

_PROG = {}


def _build_program(loop: int | None = None):
    if loop is None:
        loop = int(os.environ.get("RADON_LOOP", "0"))
    key = loop
    if key in _PROG:
        return _PROG[key]
    import concourse.bacc as bacc
    import concourse.mybir as mybir
    from concourse.tile import TileContext

    tabs, slot_angle, ext_slot, s_slot = _get_tables()

    dt16 = mybir.dt.float16
    LOOP = loop

    sh_sizes = [BATCH * 2 * int(e) for e in ext_slot]        # per-partition elems
    d_sizes = [int(s) * 2 * int(e) for s, e in zip(s_slot, ext_slot)]
    sh_off = np.concatenate([[0], np.cumsum(sh_sizes)])
    d_off = np.concatenate([[0], np.cumsum(d_sizes)])
    SH_TOT = int(sh_off[-1])
    D_TOT = int(d_off[-1])
    EXTMAX = int(ext_slot.max())
    GS_ROWS = 2 * int(s_slot.max()) * G                       # psum rows (h,s,g)

    nc = bacc.Bacc("TRN2", target_bir_lowering=False, debug=False,
                   num_devices=N_CORES)
    sh_dram = nc.dram_tensor("sh_in", [128, SH_TOT], dt16,
                             kind="ExternalInput").ap()
    d_dram = nc.dram_tensor("d_in", [128, D_TOT], dt16,
                            kind="ExternalInput").ap()
    m_dram = nc.dram_tensor("m_in", [128, 16 * NSLOT], dt16,
                            kind="ExternalInput").ap()
    NBLK = 2 * int(s_slot.max())                 # (h,s) blocks per slot
    t_dram = nc.dram_tensor("t_out", [32, NSLOT * NBLK * EXTMAX],
                            dt16, kind="ExternalOutput").ap()

    with TileContext(nc) as tc:
        with tc.tile_pool(name="res", bufs=1) as res_pool, \
             tc.tile_pool(name="shp", bufs=3) as sh_pool, \
             tc.tile_pool(name="pp", bufs=3) as p_pool, \
             tc.tile_pool(name="psum", bufs=1, space="PSUM") as psum_pool:

            # preamble: resident coefficients + masks (loaded once)
            d_res = res_pool.tile([128, D_TOT], dt16)
            m_res = res_pool.tile([128, 16 * NSLOT], dt16)
            nc.sync.dma_start(out=d_res[:], in_=d_dram[:, :])
            nc.sync.dma_start(out=m_res[:], in_=m_dram[:, :])

            def _slot_loop():
                for s in range(NSLOT):
                    E = int(ext_slot[s])
                    Ss = int(s_slot[s])
                    sh_t = sh_pool.tile([128, BATCH * 2 * EXTMAX], dt16,
                                        tag="sh")
                    nc.sync.dma_start(
                        out=sh_t[:, : sh_sizes[s]],
                        in_=sh_dram[:, sh_off[s] : sh_off[s] + sh_sizes[s]],
                    )
                    p_t = p_pool.tile([128, 2 * Ss * BATCH * EXTMAX], dt16,
                                      tag="p")
                    # multiply: P[h,s,b,e] = D[h,s,e] * SH[b,h,e]
                    # (one scalar_tensor_tensor per (h,s): verifier caps APs
                    #  at 3 dims; STT gets the 4x_2p DVE mode)
                    d4 = d_res[:, d_off[s] : d_off[s] + d_sizes[s]].rearrange(
                        "p (h s e) -> p h s e", h=2, s=Ss, e=E
                    )
                    sh4 = sh_t[:, : sh_sizes[s]].rearrange(
                        "p (b h e) -> p h b e", b=BATCH, h=2, e=E
                    )
                    p5 = p_t[:, : 2 * Ss * BATCH * E].rearrange(
                        "p (h s b e) -> p h s b e", h=2, s=Ss, b=BATCH, e=E
                    )
                    for h in range(2):
                        for q in range(Ss):
                            d3 = (d4[:, h, q].unsqueeze(1)
                                  .to_broadcast([128, BATCH, E]))
                            nc.vector.scalar_tensor_tensor(
                                out=p5[:, h, q], in0=d3, scalar=1.0,
                                in1=sh4[:, h],
                                op0=mybir.AluOpType.mult,
                                op1=mybir.AluOpType.mult,
                            )
                    # reduce rows: per (h,s,b) mask-matmul; batches live at
                    # partition bases 32*b of one psum tile per (h,s) block
                    st = p_pool.tile([128, NBLK * EXTMAX], dt16, tag="st")
                    for h in range(2):
                        lhs = m_res[:, s * 16 + h * 8 : s * 16 + h * 8 + 8]
                        for q in range(Ss):
                            ps = psum_pool.tile([128, EXTMAX],
                                                mybir.dt.float32, space="PSUM",
                                                tag=f"ps{h}{q}")
                            for b in range(BATCH):
                                nc.tensor.matmul(
                                    out=ps[32 * b : 32 * b + 8, :E],
                                    lhsT=lhs,
                                    rhs=p5[:, h, q, b, :],
                                    start=True,
                                    stop=True,
                                    tile_position=(0, 32 * b),
                                )
                            blk = h * Ss + q
                            nc.scalar.copy(
                                out=st[:, blk * E : (blk + 1) * E],
                                in_=ps[:, :E],
                            )
                    col = s * NBLK * EXTMAX
                    for u in range(BATCH):
                        nc.scalar.dma_start(
                            out=t_dram[u * 8 : (u + 1) * 8,
                                       col : col + NBLK * E],
                            in_=st[u * 32 : u * 32 + 8, : NBLK * E],
                        )

            if LOOP > 1:
                with tc.For_i(0, LOOP, 1):
                    _slot_loop()
            else:
                _slot_loop()

    nc.finalize()
    _PROG[key] = (nc, sh_off, sh_sizes, d_off, d_sizes, SH_TOT, D_TOT,
                  EXTMAX, GS_ROWS)
    return _PROG[key]


# ----------------------------------------------------------------------------
# host pack / unpack
# ----------------------------------------------------------------------------

_PACK_CONST = None


def _pack_consts():
    """Per-core constant DRAM images for d_in / m_in (cached)."""
    global _PACK_CONST
    if _PACK_CONST is not None:
        return _PACK_CONST
    tabs, slot_angle, ext_slot, s_slot = _get_tables()
    _, sh_off, sh_sizes, d_off, d_sizes, SH_TOT, D_TOT, EXTMAX, GS_ROWS = \
        _build_program(0)

    d_cores = [np.zeros((128, D_TOT), dtype=np.float16) for _ in range(N_CORES)]
    m_cores = [np.zeros((128, 16 * NSLOT), dtype=np.float16)
               for _ in range(N_CORES)]
    for s in range(NSLOT):
        E = int(ext_slot[s])
        Ss = int(s_slot[s])
        for k in range(N_CORES):
            a = slot_angle[s, k]
            if a < 0:
                continue
            tb = tabs[a]
            Ea, Sa = tb["EXT"], tb["S"]
            # D [S,256,EXT] -> [128p, (h2, Ss, E)]
            dd = np.zeros((128, 2, Ss, E), dtype=np.float16)
            Dv = tb["D"].astype(np.float16)          # [Sa,256,Ea]
            dd[:, 0, :Sa, :Ea] = Dv[:, :128].transpose(1, 0, 2)
            dd[:, 1, :Sa, :Ea] = Dv[:, 128:].transpose(1, 0, 2)
            d_cores[k][:, d_off[s] : d_off[s] + d_sizes[s]] = dd.reshape(128, -1)
            # masks [128, (h,8)]
            mm = np.zeros((128, 2, 8), dtype=np.float16)
            grp = tb["group"]
            for h in range(2):
                mm[np.arange(128), h, grp[h * 128 : (h + 1) * 128]] = 1.0
            m_cores[k][:, s * 16 : (s + 1) * 16] = mm.reshape(128, 16)
    _PACK_CONST = (d_cores, m_cores)
    return _PACK_CONST


def _host_pack(img: np.ndarray):
    """img [4,1,256,256] f32 -> per-core sh_in [128, SH_TOT] fp16."""
    tabs, slot_angle, ext_slot, s_slot = _get_tables()
    _, sh_off, sh_sizes, d_off, d_sizes, SH_TOT, D_TOT, EXTMAX, GS_ROWS = \
        _build_program(0)

    im0 = np.ascontiguousarray(img[:, 0].astype(np.float32))
    ims = [im0, np.ascontiguousarray(im0.transpose(0, 2, 1))]
    # shared padded images wide enough for all shears
    lo = min(int(tb["t"].min()) for tb in tabs)
    hi = max(int(tb["t"].max() + tb["EXT"]) for tb in tabs)
    LPAD = max(0, -lo)
    W = LPAD + max(hi, N)
    pads = []
    for v in ims:
        p = np.zeros((BATCH, N, W), dtype=np.float32)
        p[:, :, LPAD : LPAD + N] = v
        pads.append(p)

    sh_cores = [np.zeros((128, SH_TOT), dtype=np.float16)
                for _ in range(N_CORES)]
    rows = np.arange(N)[:, None]
    for s in range(NSLOT):
        E = int(ext_slot[s])
        for k in range(N_CORES):
            a = slot_angle[s, k]
            if a < 0:
                continue
            tb = tabs[a]
            Ea = tb["EXT"]
            cols = (tb["t"][:, None] + LPAD + np.arange(Ea)[None, :])
            sh = pads[tb["axis"]][:, rows, cols]          # [4,256,Ea] f32
            # layout [128p, (b, h, E)]
            out = np.zeros((128, BATCH, 2, E), dtype=np.float16)
            out[:, :, 0, :Ea] = sh[:, :128].transpose(1, 0, 2)
            out[:, :, 1, :Ea] = sh[:, 128:].transpose(1, 0, 2)
            sh_cores[k][:, sh_off[s] : sh_off[s] + sh_sizes[s]] = \
                out.reshape(128, -1)
    return sh_cores


def _unpack(results):
    """Per-core t_out [GS_ROWS, NSLOT*4*EXTMAX] f32 -> sino [4,1,180,256]."""
    tabs, slot_angle, ext_slot, s_slot = _get_tables()
    _, sh_off, sh_sizes, d_off, d_sizes, SH_TOT, D_TOT, EXTMAX, GS_ROWS = \
        _build_program(0)

    NBLK = 2 * int(s_slot.max())
    sino = np.zeros((BATCH, 1, NUM_ANGLES, NUM_DET), dtype=np.float32)
    for k in range(N_CORES):
        T = results[k]["t_out"]            # [32, NSLOT*NBLK*EXTMAX]
        for s in range(NSLOT):
            a = slot_angle[s, k]
            if a < 0:
                continue
            tb = tabs[a]
            Ea, Sa = tb["EXT"], tb["S"]
            Ss = int(s_slot[s])
            Es = int(ext_slot[s])
            c0 = s * NBLK * EXTMAX
            blk = T[:, c0 : c0 + NBLK * Es].reshape(32, NBLK, Es)
            blk = blk.reshape(BATCH, 8, NBLK, Es)[:, :, :, :Ea]
            # sum halves h: block index h*Ss+q -> [B, 8g, Sa, Ea]
            Tg = blk[:, :, 0:Sa] + blk[:, :, Ss : Ss + Sa]
            # cidx order is (g, s, xh)
            flat = np.ascontiguousarray(Tg).reshape(BATCH, -1)
            cidx = tb["cidx"]
            for b in range(BATCH):
                acc = np.bincount(cidx, weights=flat[b],
                                  minlength=NUM_DET + Sa + 1)
                sino[b, 0, a, :] = acc[:NUM_DET]
    return sino


def kernel(image: np.ndarray, _trace: bool = False):
    from concourse import bass_utils

    image = np.asarray(image)
    nc = _build_program(0)[0]
    d_cores, m_cores = _pack_consts()
    sh_cores = _host_pack(image)

    in_maps = [
        {"sh_in": sh_cores[k], "d_in": d_cores[k], "m_in": m_cores[k]}
        for k in range(N_CORES)
    ]
    res = bass_utils.run_bass_kernel_spmd(
        nc, in_maps, core_ids=list(range(N_CORES)), trace=_trace
    )
    sino = _unpack(res.results)
    if _trace:
        return sino, res
    return sino
